# revision 1
# baseline (speedup 1.0000x reference)
"""Trainium2 Bass kernel for the 2-layer GAT block (nn_GATblock_58282706206740).

Strategy (8 NeuronCores, SPMD):
  - Edges (incl. self-loops) are sharded by destination-node range: core c owns
    dst nodes [1250c, 1250(c+1)), split into 10 fixed blocks of 125 nodes.
    Per (core, block) edge lists are padded to a common per-block tile count
    (max over cores), so one program serves all cores with per-core data.
  - Per-node tables live in DRAM; per-edge rows arrive via gpsimd dma_gather
    (1024 indices per call). The layer-1 table row packs [h1 | a_src1] (1536B);
    the layer-2 table packs [h2 | a_src2] in bf16 (256B).
  - Per-tile indicator matrices S [128e x 128d] / ST are precomputed on host
    (bf16, 0/1) and streamed from DRAM. Segment softmax-sum and message
    aggregation run as one fused PE matmul per tile with rhs [exp | exp*feat];
    e_dst expands via an ST x a_dst matmul. The softmax skips the max
    subtraction (scores are provably < ~4, exp is safe in fp32) and keeps the
    reference's +1e-16 denominator epsilon.
  - Between layers, one AllGather exchanges the bf16 [h2 | a_src2] node table
    (the only cross-core traffic).
"""
import sys

sys.path.insert(0, "/opt/trn_rl_repo")

import ml_dtypes
import numpy as np

N_NODES = 10000
N_CORES = 8
NPC = N_NODES // N_CORES          # 1250
B_BLOCKS = 10
NPB = NPC // B_BLOCKS             # 125
TILE_E = 128
CHUNK = 8
PAD_COL = 200.0
EPS = 1e-16
NEG_SLOPE = 0.2
F0, F1, F2, H1 = 128, 320, 64, 5
ROW1, ROW2 = 384, 128


def _build_partition(edge_index):
    src = np.concatenate([edge_index[0].astype(np.int64),
                          np.arange(N_NODES, dtype=np.int64)])
    dst = np.concatenate([edge_index[1].astype(np.int64),
                          np.arange(N_NODES, dtype=np.int64)])
    core = dst // NPC
    block = (dst % NPC) // NPB
    col = dst % NPB

    cnt = np.zeros((N_CORES, B_BLOCKS), dtype=np.int64)
    np.add.at(cnt, (core, block), 1)
    T_b = np.ceil(cnt.max(axis=0) / TILE_E).astype(np.int64)
    tile_ofs = np.concatenate([[0], np.cumsum(T_b)])
    Ttot = int(tile_ofs[-1])
    Epad = Ttot * TILE_E

    src_sl = np.zeros((N_CORES, Epad), dtype=np.int64)
    col_sl = np.full((N_CORES, Epad), PAD_COL, dtype=np.float32)
    order = np.lexsort((dst, core * B_BLOCKS + block))
    s_src, s_core, s_block, s_col = src[order], core[order], block[order], col[order]
    idx = 0
    for c in range(N_CORES):
        for b in range(B_BLOCKS):
            n = int(cnt[c, b])
            base = int(tile_ofs[b]) * TILE_E
            sl = slice(idx, idx + n)
            assert np.all(s_core[sl] == c) and np.all(s_block[sl] == b)
            src_sl[c, base:base + n] = s_src[sl]
            col_sl[c, base:base + n] = s_col[sl]
            idx += n
    assert idx == len(src)
    return src_sl, col_sl, tile_ofs, Ttot, Epad


def _wrap_idx16(idx):
    a = idx.astype(np.int16).reshape(-1, 16).T
    return np.tile(a, (8, 1))


def _host_prep(inputs):
    x = np.asarray(inputs["x"], dtype=np.float32)
    W1 = np.asarray(inputs["W1"], dtype=np.float32)
    att_src1 = np.asarray(inputs["att_src1"], dtype=np.float32)
    att_dst1 = np.asarray(inputs["att_dst1"], dtype=np.float32)
    b1 = np.asarray(inputs["b1"], dtype=np.float32)
    W2 = np.asarray(inputs["W2"], dtype=np.float32)
    att_src2 = np.asarray(inputs["att_src2"], dtype=np.float32)
    att_dst2 = np.asarray(inputs["att_dst2"], dtype=np.float32)
    b2 = np.asarray(inputs["b2"], dtype=np.float32)
    ei = np.asarray(inputs["edge_index"])

    src_sl, col_sl, tile_ofs, Ttot, Epad = _build_partition(ei)

    A1 = np.zeros((F1, 2 * H1), dtype=np.float32)
    for h in range(H1):
        A1[64 * h:64 * h + 64, h] = att_src1[h]
        A1[64 * h:64 * h + 64, H1 + h] = att_dst1[h]
    W1A1 = (W1 @ A1).astype(np.float32)
    A2 = np.stack([att_src2[0], att_dst2[0]], axis=1).astype(np.float32)
    W2A2 = (W2 @ A2).astype(np.float32)

    xT = np.ascontiguousarray(x.T)
    shared = dict(
        xT=xT,
        W1cat=np.concatenate([W1, W1A1], axis=1),
        W1A1=W1A1, W2=W2, W2A2=W2A2,
        ident=np.eye(128, dtype=np.float32),
        b1rep=np.broadcast_to(b1, (128, F1)).copy(),
        b2rep=np.broadcast_to(b2, (128, F2)).copy(),
    )
    d = np.arange(128, dtype=np.float32)
    per_core = []
    for c in range(N_CORES):
        colf = np.ascontiguousarray(col_sl[c].reshape(Ttot, TILE_E).T)
        S = (colf[:, :, None] == d[None, None, :])
        per_core.append(dict(
            src16=_wrap_idx16(src_sl[c]),
            Sb=np.ascontiguousarray(S).astype(ml_dtypes.bfloat16),
            STb=np.ascontiguousarray(np.transpose(S, (2, 1, 0))).astype(ml_dtypes.bfloat16),
            xTc=np.ascontiguousarray(xT[:, c * NPC:(c + 1) * NPC]),
        ))
    return shared, per_core, tile_ofs, Ttot, Epad


def _build_program(tile_ofs, Ttot, Epad):
    import concourse.bacc as bacc
    import concourse.mybir as mybir
    from concourse import tile

    dt = mybir.dt
    F32 = dt.float32
    BF16 = dt.bfloat16
    AF = mybir.ActivationFunctionType
    OP = mybir.AluOpType

    N = N_NODES
    B = B_BLOCKS
    tile_ofs = [int(v) for v in tile_ofs]
    block_of_tile = np.zeros(Ttot, dtype=np.int64)
    for b in range(B):
        block_of_tile[tile_ofs[b]:tile_ofs[b + 1]] = b
    n_node_tiles = (N + 127) // 128
    n_chunks = (Ttot + CHUNK - 1) // CHUNK

    nc = bacc.Bacc("TRN2", target_bir_lowering=False, debug=False,
                   num_devices=N_CORES)

    xT_d = nc.dram_tensor("xT", [F0, N], F32, kind="ExternalInput")
    xTc_d = nc.dram_tensor("xTc", [F0, NPC], F32, kind="ExternalInput")
    W1c_d = nc.dram_tensor("W1cat", [F0, F1 + 2 * H1], F32, kind="ExternalInput")
    W1A1_d = nc.dram_tensor("W1A1", [F0, 2 * H1], F32, kind="ExternalInput")
    W2_d = nc.dram_tensor("W2", [F1, F2], F32, kind="ExternalInput")
    W2A2_d = nc.dram_tensor("W2A2", [F1, 2], F32, kind="ExternalInput")
    ident_d = nc.dram_tensor("ident", [128, 128], F32, kind="ExternalInput")
    b1_d = nc.dram_tensor("b1rep", [128, F1], F32, kind="ExternalInput")
    b2_d = nc.dram_tensor("b2rep", [128, F2], F32, kind="ExternalInput")
    src16_d = nc.dram_tensor("src16", [128, Epad // 16], dt.int16, kind="ExternalInput")
    S_d = nc.dram_tensor("Sb", [128, Ttot, 128], BF16, kind="ExternalInput")
    ST_d = nc.dram_tensor("STb", [128, Ttot, 128], BF16, kind="ExternalInput")
    out_d = nc.dram_tensor("out", [NPC, F2], F32, kind="ExternalOutput")

    with tile.TileContext(nc) as tc:
        with (
            tc.tile_pool(name="dram", bufs=1, space="DRAM") as dram,
            tc.tile_pool(name="const", bufs=1) as cpool,
        ):
            h1tab = dram.tile([N, ROW1], F32)
            ag_in = dram.tile([NPC, ROW2], BF16)
            h2tab = dram.tile([N, ROW2], BF16, addr_space="Shared")

            W1cs = cpool.tile([F0, F1 + 2 * H1], F32)
            nc.sync.dma_start(W1cs[:], W1c_d[:])
            W2s = cpool.tile([128, 3, F2], F32)
            W2A2s = cpool.tile([128, 3, 2], F32)
            for k in range(3):
                w3 = min(128, F1 - 128 * k)
                nc.sync.dma_start(W2s[:w3, k, :], W2_d[128 * k:128 * k + w3, :])
                nc.sync.dma_start(W2A2s[:w3, k, :], W2A2_d[128 * k:128 * k + w3, :])
            idents = cpool.tile([128, 128], F32)
            nc.sync.dma_start(idents[:], ident_d[:])
            b1s = cpool.tile([128, F1], F32)
            nc.sync.dma_start(b1s[:], b1_d[:])
            b2s = cpool.tile([128, F2], F32)
            nc.sync.dma_start(b2s[:], b2_d[:])
            src16 = cpool.tile([128, Epad // 16], dt.int16)
            nc.sync.dma_start(src16[:], src16_d[:])
            adst1 = [cpool.tile([128, H1], BF16, name=f"adst1_{b}") for b in range(B)]
            adst2 = [cpool.tile([128, 1], BF16, name=f"adst2_{b}") for b in range(B)]

            # ---------------- phase A: h1/a1 node tables (replicated) ----
            with (
                tc.tile_pool(name="pA", bufs=4) as pA,
                tc.tile_pool(name="pAx", bufs=1) as pAx,
                tc.tile_pool(name="psA", bufs=2, space="PSUM") as psA,
            ):
                xTcs = pA.tile([F0, NPC], F32, tag="xTc", bufs=1)
                nc.sync.dma_start(xTcs[:], xTc_d[:])
                for b in range(B):
                    pb = psA.tile([128, H1], F32, tag="pb")
                    nc.tensor.matmul(pb[:NPB, :], xTcs[:, NPB * b:NPB * (b + 1)],
                                     W1cs[:, F1 + H1:F1 + 2 * H1], start=True, stop=True)
                    nc.vector.tensor_copy(adst1[b][:NPB, :], pb[:NPB, :])
                PART = 13 * 128
                xparts = []
                for p in range(0, N, PART):
                    wp = min(PART, N - p)
                    xp = pAx.tile([F0, wp], F32, name=f"xp{p}")
                    nc.sync.dma_start(xp[:], xT_d[:, p:p + wp])
                    xparts.append(xp)
                for nt in range(n_node_tiles):
                    w = min(128, N - 128 * nt)
                    pi, po = divmod(128 * nt, PART)
                    lhs = xparts[pi][:, po:po + w]
                    ph = psA.tile([128, F1 + 2 * H1], F32, tag="ph")
                    nc.tensor.matmul(ph[:w, :], lhs, W1cs[:], start=True, stop=True)
                    row = pA.tile([128, ROW1], F32, tag="row")
                    nc.vector.memset(row[:w, F1 + H1:ROW1], 0.0)
                    nc.vector.tensor_copy(row[:w, 0:F1 + H1], ph[:w, 0:F1 + H1])
                    nc.sync.dma_start(h1tab[128 * nt:128 * nt + w, :], row[:w, :])

            # ---------------- shared edge-layer runner -------------------
            def edge_layer(tab, F, H, ROWT, adst, bias, out_sink, gdt=F32):
                C = F // H
                with (
                    tc.tile_pool(name="gbuf", bufs=2) as gbuf,
                    tc.tile_pool(name="sbuf2", bufs=3) as sb2,
                    tc.tile_pool(name="ps_ed", bufs=2, space="PSUM") as ps_ed,
                    tc.tile_pool(name="ps_acc", bufs=2, space="PSUM") as ps_acc,
                    tc.tile_pool(name="ps_epi", bufs=2, space="PSUM") as ps_epi,
                    tc.tile_pool(name="epi", bufs=2) as epi,
                ):
                    usp = None
                    for ch in range(n_chunks):
                        t0 = ch * CHUNK
                        tn = min(CHUNK, Ttot - t0)
                        g = gbuf.tile([128, tn, ROWT], gdt, tag="g", bufs=4)
                        nc.gpsimd.dma_gather(
                            g[:], tab[:], src16[:, t0 * 8:t0 * 8 + tn * 8],
                            num_idxs=tn * 128, num_idxs_reg=tn * 128,
                            elem_size=ROWT)
                        Ssl = gbuf.tile([128, tn, 128], BF16, tag="Ssl", bufs=2)
                        nc.sync.dma_start(Ssl[:], S_d[:, t0:t0 + tn, :])
                        STsl = gbuf.tile([128, tn, 128], BF16, tag="STsl", bufs=2)
                        nc.sync.dma_start(STsl[:], ST_d[:, t0:t0 + tn, :])

                        edp = ps_ed.tile([128, tn, H], F32, tag="edp")
                        for tl in range(tn):
                            b = int(block_of_tile[t0 + tl])
                            nc.tensor.matmul(edp[:, tl, :], STsl[:NPB, tl, :],
                                             adst[b][:NPB, :], start=True, stop=True)
                        es = sb2.tile([128, tn, H], F32, tag="es", bufs=3)
                        nc.vector.tensor_tensor(
                            es[:], g[:, :, F:F + H], edp[:], OP.add)
                        nc.vector.scalar_tensor_tensor(
                            es[:], es[:], NEG_SLOPE, es[:], OP.mult, OP.max)
                        exhs = sb2.tile([128, tn, H + F], BF16, tag="exhs", bufs=3)
                        nc.scalar.activation(exhs[:, :, 0:H], es[:], AF.Exp)
                        g4 = g[:, :, 0:F].rearrange("p t (h c) -> p t h c", h=H)
                        hs4 = exhs[:, :, H:H + F].rearrange("p t (h c) -> p t h c", h=H)
                        ex4 = exhs[:, :, 0:H].unsqueeze(3).broadcast_to((128, tn, H, C))
                        nc.vector.tensor_tensor(hs4, g4, ex4, OP.mult)

                        for tl in range(tn):
                            t = t0 + tl
                            b = int(block_of_tile[t])
                            first = t == tile_ofs[b]
                            last = t == tile_ofs[b + 1] - 1
                            if first:
                                usp = ps_acc.tile([128, H + F], F32, tag="usp")
                            nc.tensor.matmul(usp[:NPB, :], Ssl[:, tl, 0:NPB],
                                             exhs[:, tl, :], start=first, stop=last)
                            if last:
                                rec = epi.tile([128, H], F32, tag="rec")
                                nc.vector.tensor_scalar_add(rec[:NPB, :], usp[:NPB, 0:H], EPS)
                                nc.vector.reciprocal(rec[:NPB, :], rec[:NPB, :])
                                o1 = epi.tile([128, F], F32, tag="o1")
                                for h in range(H):
                                    nc.vector.scalar_tensor_tensor(
                                        o1[:NPB, C * h:C * h + C],
                                        usp[:NPB, H + C * h:H + C * h + C],
                                        rec[:NPB, h:h + 1],
                                        bias[:NPB, C * h:C * h + C],
                                        OP.mult, OP.add)
                                o1r = epi.tile([128, F], F32, tag="o1r")
                                nc.scalar.activation(o1r[:NPB, :], o1[:NPB, :], AF.Relu)
                                out_sink(b, o1r, ps_epi, epi)

            # ---------------- layer 1 + inter-layer prep -----------------
            def sink1(b, o1r, ps_epi, epi):
                h1T = epi.tile([128, 3, NPB], F32, tag="h1T")
                for k in range(3):
                    w3 = min(128, F1 - 128 * k)
                    tp = ps_epi.tile([128, NPB], F32, tag="tp")
                    nc.tensor.transpose(tp[:w3, :], o1r[:NPB, 128 * k:128 * k + w3],
                                        idents[:NPB, :NPB])
                    nc.vector.tensor_copy(h1T[:w3, k, :], tp[:w3, :])
                h2ps = ps_epi.tile([128, F2 + 2], F32, tag="h2ps")
                for k in range(3):
                    w3 = min(128, F1 - 128 * k)
                    nc.tensor.matmul(h2ps[:NPB, 0:F2], h1T[:w3, k, :],
                                     W2s[:w3, k, :], start=(k == 0), stop=(k == 2))
                for k in range(3):
                    w3 = min(128, F1 - 128 * k)
                    nc.tensor.matmul(h2ps[:NPB, F2:F2 + 2], h1T[:w3, k, :],
                                     W2A2s[:w3, k, :], start=(k == 0), stop=(k == 2))
                agrow = epi.tile([128, ROW2], BF16, tag="agrow")
                nc.vector.memset(agrow[:NPB, F2 + 1:ROW2], 0.0)
                nc.vector.tensor_copy(agrow[:NPB, 0:F2 + 1], h2ps[:NPB, 0:F2 + 1])
                nc.sync.dma_start(ag_in[NPB * b:NPB * (b + 1), :], agrow[:NPB, :])
                nc.vector.tensor_copy(adst2[b][:NPB, :], h2ps[:NPB, F2 + 1:F2 + 2])

            edge_layer(h1tab, F1, H1, ROW1, adst1, b1s, sink1)

            import concourse.mybir as _mb
            nc.gpsimd.collective_compute(
                "AllGather", OP.bypass, replica_groups=[list(range(N_CORES))],
                ins=[ag_in.opt()], outs=[h2tab.opt()])

            def sink2(b, o2r, ps_epi, epi):
                nc.sync.dma_start(out_d[NPB * b:NPB * (b + 1), :], o2r[:NPB, 0:F2])

            edge_layer(h2tab, F2, 1, ROW2, adst2, b2s, sink2, gdt=BF16)

    nc.compile()
    return nc


def kernel(**inputs) -> np.ndarray:
    import time

    from concourse.bass_utils import run_bass_kernel_spmd

    shared, per_core, tile_ofs, Ttot, Epad = _host_prep(inputs)
    nc = _build_program(tile_ofs, Ttot, Epad)

    in_maps = []
    for c in range(N_CORES):
        m = dict(shared)
        m.update(per_core[c])
        in_maps.append(m)
    res = None
    for attempt in range(3):
        try:
            res = run_bass_kernel_spmd(nc, in_maps, list(range(N_CORES)))
            break
        except Exception:
            if attempt == 2:
                raise
            time.sleep(5)
    out = np.concatenate([res.results[c]["out"] for c in range(N_CORES)], axis=0)
    return np.ascontiguousarray(out.astype(np.float32))



# revision 5
# speedup vs baseline: 1.2857x; 1.2857x over previous
"""Trainium2 Bass kernel for the 2-layer GAT block (nn_GATblock_58282706206740).

Strategy (8 NeuronCores, SPMD):
  - Edges (incl. self-loops) are sharded by destination-node range: core c owns
    dst nodes [1250c, 1250(c+1)), split into 10 fixed blocks of 125 nodes.
    Per (core, block) edge lists are padded to a common per-block tile count
    (max over cores), so one program serves all cores with per-core data.
  - Per-node tables live in DRAM in bf16; per-edge rows arrive via gpsimd
    dma_gather (CHUNK tiles x 128 indices per call). The layer-1 table row
    packs [h1 | a_src1] as 384 bf16 (768B); the layer-2 table packs
    [h2 | a_src2] as 128 bf16 (256B).
  - Per-tile indicator matrices S [128e x 128d] / ST are precomputed on host
    in fp8e4 (exact 0/1) and kept RESIDENT in SBUF across both layers
    (~10.9 MB), removing the per-chunk S/ST streams entirely. Segment
    softmax-sum and message aggregation run as one fused PE matmul per tile
    with rhs [exp | exp*feat] (bf16 moving, fp8 stationary); e_dst expands via
    an ST x a_dst matmul. The softmax skips the max subtraction (scores are
    provably < ~4, exp is safe) and keeps the reference's +1e-16 epsilon.
  - Between layers, one AllGather exchanges the bf16 [h2 | a_src2] node table
    (the only cross-core traffic).
"""
import sys

sys.path.insert(0, "/opt/trn_rl_repo")

import ml_dtypes
import numpy as np

N_NODES = 10000
N_CORES = 8
NPC = N_NODES // N_CORES          # 1250
B_BLOCKS = 10
NPB = NPC // B_BLOCKS             # 125
TILE_E = 128
CHUNK = 8          # dma_gather ucode wedges above 1024 indices per call
N_QUEUES = 2       # alternate SWDGE queues -> gather desc-gen runs on two Q7 pairs
PAD_COL = 200.0
EPS = 1e-16
NEG_SLOPE = 0.2
F0, F1, F2, H1 = 128, 320, 64, 5
ROW1, ROW2 = 384, 128             # bf16 elements per table row


def _build_partition(edge_index):
    src = np.concatenate([edge_index[0].astype(np.int64),
                          np.arange(N_NODES, dtype=np.int64)])
    dst = np.concatenate([edge_index[1].astype(np.int64),
                          np.arange(N_NODES, dtype=np.int64)])
    core = dst // NPC
    block = (dst % NPC) // NPB
    col = dst % NPB

    cnt = np.zeros((N_CORES, B_BLOCKS), dtype=np.int64)
    np.add.at(cnt, (core, block), 1)
    T_b = np.ceil(cnt.max(axis=0) / TILE_E).astype(np.int64)
    tile_ofs = np.concatenate([[0], np.cumsum(T_b)])
    Ttot = int(tile_ofs[-1])
    Epad = Ttot * TILE_E

    src_sl = np.zeros((N_CORES, Epad), dtype=np.int64)
    col_sl = np.full((N_CORES, Epad), PAD_COL, dtype=np.float32)
    order = np.lexsort((dst, core * B_BLOCKS + block))
    s_src, s_core, s_block, s_col = src[order], core[order], block[order], col[order]
    idx = 0
    for c in range(N_CORES):
        for b in range(B_BLOCKS):
            n = int(cnt[c, b])
            base = int(tile_ofs[b]) * TILE_E
            sl = slice(idx, idx + n)
            assert np.all(s_core[sl] == c) and np.all(s_block[sl] == b)
            src_sl[c, base:base + n] = s_src[sl]
            col_sl[c, base:base + n] = s_col[sl]
            idx += n
    assert idx == len(src)
    return src_sl, col_sl, tile_ofs, Ttot, Epad


def _wrap_idx16(idx):
    a = idx.astype(np.int16).reshape(-1, 16).T
    return np.tile(a, (8, 1))


def _host_prep(inputs):
    x = np.asarray(inputs["x"], dtype=np.float32)
    W1 = np.asarray(inputs["W1"], dtype=np.float32)
    att_src1 = np.asarray(inputs["att_src1"], dtype=np.float32)
    att_dst1 = np.asarray(inputs["att_dst1"], dtype=np.float32)
    b1 = np.asarray(inputs["b1"], dtype=np.float32)
    W2 = np.asarray(inputs["W2"], dtype=np.float32)
    att_src2 = np.asarray(inputs["att_src2"], dtype=np.float32)
    att_dst2 = np.asarray(inputs["att_dst2"], dtype=np.float32)
    b2 = np.asarray(inputs["b2"], dtype=np.float32)
    ei = np.asarray(inputs["edge_index"])

    src_sl, col_sl, tile_ofs, Ttot, Epad = _build_partition(ei)

    A1 = np.zeros((F1, 2 * H1), dtype=np.float32)
    for h in range(H1):
        A1[64 * h:64 * h + 64, h] = att_src1[h]
        A1[64 * h:64 * h + 64, H1 + h] = att_dst1[h]
    W1A1 = (W1 @ A1).astype(np.float32)
    A2 = np.stack([att_src2[0], att_dst2[0]], axis=1).astype(np.float32)
    W2A2 = (W2 @ A2).astype(np.float32)

    xT = np.ascontiguousarray(x.T)
    shared = dict(
        xT16=xT.astype(ml_dtypes.bfloat16),
        W1c16=np.concatenate([W1, W1A1], axis=1).astype(ml_dtypes.bfloat16),
        W2=W2, W2A2=W2A2,
        ident=np.eye(128, dtype=np.float32),
        b1rep=np.broadcast_to(b1, (128, F1)).copy(),
        b2rep=np.broadcast_to(b2, (128, F2)).copy(),
    )
    d = np.arange(128, dtype=np.float32)
    per_core = []
    for c in range(N_CORES):
        colf = np.ascontiguousarray(col_sl[c].reshape(Ttot, TILE_E).T)
        S = (colf[:, :, None] == d[None, None, :])
        per_core.append(dict(
            src16=_wrap_idx16(src_sl[c]),
            S8=np.ascontiguousarray(S).astype(ml_dtypes.float8_e4m3),
            ST8=np.ascontiguousarray(np.transpose(S, (2, 1, 0))).astype(ml_dtypes.float8_e4m3),
            xTc16=np.ascontiguousarray(
                xT[:, c * NPC:(c + 1) * NPC]).astype(ml_dtypes.bfloat16),
        ))
    return shared, per_core, tile_ofs, Ttot, Epad


def _build_program(tile_ofs, Ttot, Epad):
    import concourse.bacc as bacc
    import concourse.mybir as mybir
    from concourse import tile

    dt = mybir.dt
    F32 = dt.float32
    BF16 = dt.bfloat16
    FP8 = dt.float8e4
    AF = mybir.ActivationFunctionType
    OP = mybir.AluOpType

    N = N_NODES
    B = B_BLOCKS
    tile_ofs = [int(v) for v in tile_ofs]
    block_of_tile = np.zeros(Ttot, dtype=np.int64)
    for b in range(B):
        block_of_tile[tile_ofs[b]:tile_ofs[b + 1]] = b
    n_node_tiles = (N + 127) // 128
    n_chunks = (Ttot + CHUNK - 1) // CHUNK

    nc = bacc.Bacc("TRN2", target_bir_lowering=False, debug=False,
                   num_devices=N_CORES, num_swdge_queues=N_QUEUES)

    xT_d = nc.dram_tensor("xT16", [F0, N], BF16, kind="ExternalInput")
    xTc_d = nc.dram_tensor("xTc16", [F0, NPC], BF16, kind="ExternalInput")
    W1c_d = nc.dram_tensor("W1c16", [F0, F1 + 2 * H1], BF16, kind="ExternalInput")
    W2_d = nc.dram_tensor("W2", [F1, F2], F32, kind="ExternalInput")
    W2A2_d = nc.dram_tensor("W2A2", [F1, 2], F32, kind="ExternalInput")
    ident_d = nc.dram_tensor("ident", [128, 128], F32, kind="ExternalInput")
    b1_d = nc.dram_tensor("b1rep", [128, F1], F32, kind="ExternalInput")
    b2_d = nc.dram_tensor("b2rep", [128, F2], F32, kind="ExternalInput")
    src16_d = nc.dram_tensor("src16", [128, Epad // 16], dt.int16, kind="ExternalInput")
    S_d = nc.dram_tensor("S8", [128, Ttot, 128], FP8, kind="ExternalInput")
    ST_d = nc.dram_tensor("ST8", [128, Ttot, 128], FP8, kind="ExternalInput")
    out_d = nc.dram_tensor("out", [NPC, F2], F32, kind="ExternalOutput")

    with tile.TileContext(nc) as tc:
        with (
            tc.tile_pool(name="dram", bufs=1, space="DRAM") as dram,
            tc.tile_pool(name="const", bufs=1) as cpool,
        ):
            h1tab = dram.tile([N, ROW1], BF16)
            ag_in = dram.tile([NPC, ROW2], BF16)
            h2tab = dram.tile([N, ROW2], BF16, addr_space="Shared")

            # resident indicator matrices (fp8, both layers)
            S_res = cpool.tile([128, Ttot, 128], FP8)
            nc.sync.dma_start(S_res[:], S_d[:])
            ST_res = cpool.tile([128, Ttot, 128], FP8)
            nc.sync.dma_start(ST_res[:], ST_d[:])

            W1cs = cpool.tile([F0, F1 + 2 * H1], BF16)
            nc.sync.dma_start(W1cs[:], W1c_d[:])
            W2s = cpool.tile([128, 3, F2], F32)
            W2A2s = cpool.tile([128, 3, 2], F32)
            for k in range(3):
                w3 = min(128, F1 - 128 * k)
                nc.sync.dma_start(W2s[:w3, k, :], W2_d[128 * k:128 * k + w3, :])
                nc.sync.dma_start(W2A2s[:w3, k, :], W2A2_d[128 * k:128 * k + w3, :])
            idents = cpool.tile([128, 128], F32)
            nc.sync.dma_start(idents[:], ident_d[:])
            b1s = cpool.tile([128, F1], F32)
            nc.sync.dma_start(b1s[:], b1_d[:])
            b2s = cpool.tile([128, F2], F32)
            nc.sync.dma_start(b2s[:], b2_d[:])
            src16 = cpool.tile([128, Epad // 16], dt.int16)
            nc.sync.dma_start(src16[:], src16_d[:])
            adst1 = [cpool.tile([128, H1], BF16, name=f"adst1_{b}") for b in range(B)]
            adst2 = [cpool.tile([128, 1], BF16, name=f"adst2_{b}") for b in range(B)]

            # ---------------- phase A: h1/a1 node tables (replicated) ----
            with (
                tc.tile_pool(name="pA", bufs=4) as pA,
                tc.tile_pool(name="pAx", bufs=1) as pAx,
                tc.tile_pool(name="psA", bufs=2, space="PSUM") as psA,
            ):
                xTcs = pA.tile([F0, NPC], BF16, tag="xTc", bufs=1)
                nc.sync.dma_start(xTcs[:], xTc_d[:])
                for b in range(B):
                    pb = psA.tile([128, H1], F32, tag="pb")
                    nc.tensor.matmul(pb[:NPB, :], xTcs[:, NPB * b:NPB * (b + 1)],
                                     W1cs[:, F1 + H1:F1 + 2 * H1], start=True, stop=True)
                    nc.vector.tensor_copy(adst1[b][:NPB, :], pb[:NPB, :])
                PART = 13 * 128
                xparts = []
                for p in range(0, N, PART):
                    wp = min(PART, N - p)
                    xp = pAx.tile([F0, wp], BF16, name=f"xp{p}")
                    nc.sync.dma_start(xp[:], xT_d[:, p:p + wp])
                    xparts.append(xp)
                for nt in range(n_node_tiles):
                    w = min(128, N - 128 * nt)
                    pi, po = divmod(128 * nt, PART)
                    lhs = xparts[pi][:, po:po + w]
                    ph = psA.tile([128, F1 + 2 * H1], F32, tag="ph")
                    nc.tensor.matmul(ph[:w, :], lhs, W1cs[:], start=True, stop=True)
                    row = pA.tile([128, ROW1], BF16, tag="row")
                    nc.vector.memset(row[:w, F1 + H1:ROW1], 0.0)
                    nc.vector.tensor_copy(row[:w, 0:F1 + H1], ph[:w, 0:F1 + H1])
                    nc.sync.dma_start(h1tab[128 * nt:128 * nt + w, :], row[:w, :])

            # ---------------- shared edge-layer runner -------------------
            def edge_layer(tab, F, H, ROWT, adst, bias, out_sink):
                C = F // H
                with (
                    tc.tile_pool(name="gbuf", bufs=2) as gbuf,
                    tc.tile_pool(name="sbuf2", bufs=3) as sb2,
                    tc.tile_pool(name="ps_ed", bufs=2, space="PSUM") as ps_ed,
                    tc.tile_pool(name="ps_acc", bufs=2, space="PSUM") as ps_acc,
                    tc.tile_pool(name="ps_epi", bufs=2, space="PSUM") as ps_epi,
                    tc.tile_pool(name="epi", bufs=2) as epi,
                ):
                    usp = None
                    for ch in range(n_chunks):
                        t0 = ch * CHUNK
                        tn = min(CHUNK, Ttot - t0)
                        g = gbuf.tile([128, tn, ROWT], BF16, tag="g", bufs=4)
                        nc.gpsimd.dma_gather(
                            g[:], tab[:], src16[:, t0 * 8:t0 * 8 + tn * 8],
                            num_idxs=tn * 128, num_idxs_reg=tn * 128,
                            elem_size=ROWT, queue_num=ch % N_QUEUES)

                        edp = ps_ed.tile([128, tn, H], F32, tag="edp")
                        for tl in range(tn):
                            t = t0 + tl
                            b = int(block_of_tile[t])
                            nc.tensor.matmul(edp[:, tl, :], ST_res[:NPB, t, :],
                                             adst[b][:NPB, :], start=True, stop=True)
                        es = sb2.tile([128, tn, H], F32, tag="es", bufs=3)
                        nc.vector.tensor_tensor(
                            es[:], g[:, :, F:F + H], edp[:], OP.add)
                        nc.vector.scalar_tensor_tensor(
                            es[:], es[:], NEG_SLOPE, es[:], OP.mult, OP.max)
                        exhs = sb2.tile([128, tn, H + F], BF16, tag="exhs", bufs=3)
                        nc.scalar.activation(exhs[:, :, 0:H], es[:], AF.Exp)
                        g4 = g[:, :, 0:F].rearrange("p t (h c) -> p t h c", h=H)
                        hs4 = exhs[:, :, H:H + F].rearrange("p t (h c) -> p t h c", h=H)
                        ex4 = exhs[:, :, 0:H].unsqueeze(3).broadcast_to((128, tn, H, C))
                        nc.vector.tensor_tensor(hs4, g4, ex4, OP.mult)

                        for tl in range(tn):
                            t = t0 + tl
                            b = int(block_of_tile[t])
                            first = t == tile_ofs[b]
                            last = t == tile_ofs[b + 1] - 1
                            if first:
                                usp = ps_acc.tile([128, H + F], F32, tag="usp")
                            nc.tensor.matmul(usp[:NPB, :], S_res[:, t, 0:NPB],
                                             exhs[:, tl, :], start=first, stop=last)
                            if last:
                                rec = epi.tile([128, H], F32, tag="rec")
                                nc.vector.tensor_scalar_add(rec[:NPB, :], usp[:NPB, 0:H], EPS)
                                nc.vector.reciprocal(rec[:NPB, :], rec[:NPB, :])
                                o1 = epi.tile([128, F], F32, tag="o1")
                                for h in range(H):
                                    nc.vector.scalar_tensor_tensor(
                                        o1[:NPB, C * h:C * h + C],
                                        usp[:NPB, H + C * h:H + C * h + C],
                                        rec[:NPB, h:h + 1],
                                        bias[:NPB, C * h:C * h + C],
                                        OP.mult, OP.add)
                                o1r = epi.tile([128, F], F32, tag="o1r")
                                nc.scalar.activation(o1r[:NPB, :], o1[:NPB, :], AF.Relu)
                                out_sink(b, o1r, ps_epi, epi)

            # ---------------- layer 1 + inter-layer prep -----------------
            def sink1(b, o1r, ps_epi, epi):
                h1T = epi.tile([128, 3, NPB], F32, tag="h1T")
                for k in range(3):
                    w3 = min(128, F1 - 128 * k)
                    tp = ps_epi.tile([128, NPB], F32, tag="tp")
                    nc.tensor.transpose(tp[:w3, :], o1r[:NPB, 128 * k:128 * k + w3],
                                        idents[:NPB, :NPB])
                    nc.vector.tensor_copy(h1T[:w3, k, :], tp[:w3, :])
                h2ps = ps_epi.tile([128, F2 + 2], F32, tag="h2ps")
                for k in range(3):
                    w3 = min(128, F1 - 128 * k)
                    nc.tensor.matmul(h2ps[:NPB, 0:F2], h1T[:w3, k, :],
                                     W2s[:w3, k, :], start=(k == 0), stop=(k == 2))
                for k in range(3):
                    w3 = min(128, F1 - 128 * k)
                    nc.tensor.matmul(h2ps[:NPB, F2:F2 + 2], h1T[:w3, k, :],
                                     W2A2s[:w3, k, :], start=(k == 0), stop=(k == 2))
                agrow = epi.tile([128, ROW2], BF16, tag="agrow")
                nc.vector.memset(agrow[:NPB, F2 + 1:ROW2], 0.0)
                nc.vector.tensor_copy(agrow[:NPB, 0:F2 + 1], h2ps[:NPB, 0:F2 + 1])
                nc.sync.dma_start(ag_in[NPB * b:NPB * (b + 1), :], agrow[:NPB, :])
                nc.vector.tensor_copy(adst2[b][:NPB, :], h2ps[:NPB, F2 + 1:F2 + 2])

            edge_layer(h1tab, F1, H1, ROW1, adst1, b1s, sink1)

            nc.gpsimd.collective_compute(
                "AllGather", OP.bypass, replica_groups=[list(range(N_CORES))],
                ins=[ag_in.opt()], outs=[h2tab.opt()])

            def sink2(b, o2r, ps_epi, epi):
                nc.sync.dma_start(out_d[NPB * b:NPB * (b + 1), :], o2r[:NPB, 0:F2])

            edge_layer(h2tab, F2, 1, ROW2, adst2, b2s, sink2)

    nc.compile()
    return nc


def kernel(**inputs) -> np.ndarray:
    import time

    from concourse.bass_utils import run_bass_kernel_spmd

    shared, per_core, tile_ofs, Ttot, Epad = _host_prep(inputs)
    nc = _build_program(tile_ofs, Ttot, Epad)

    in_maps = []
    for c in range(N_CORES):
        m = dict(shared)
        m.update(per_core[c])
        in_maps.append(m)
    res = None
    for attempt in range(3):
        try:
            res = run_bass_kernel_spmd(nc, in_maps, list(range(N_CORES)))
            break
        except Exception:
            if attempt == 2:
                raise
            time.sleep(5)
    out = np.concatenate([res.results[c]["out"] for c in range(N_CORES)], axis=0)
    return np.ascontiguousarray(out.astype(np.float32))


# revision 8
# speedup vs baseline: 1.5783x; 1.2276x over previous
"""Trainium2 Bass kernel for the 2-layer GAT block (nn_GATblock_58282706206740).

Strategy (8 NeuronCores, SPMD):
  - Edges (incl. self-loops) are sharded by destination-node range: core c owns
    dst nodes [1250c, 1250(c+1)), split into 10 fixed blocks of 125 nodes.
    Per (core, block) edge lists are padded to a common per-block tile count
    (max over cores), so one program serves all cores with per-core data.
  - Per-node tables live in DRAM in bf16; per-edge rows arrive via gpsimd
    dma_gather (CHUNK tiles x 128 indices per call). The layer-1 table row
    packs [h1 | a_src1] as 384 bf16 (768B); the layer-2 table packs
    [h2 | a_src2] as 128 bf16 (256B).
  - Per-tile indicator matrices S [128e x 128d] / ST are precomputed on host
    in fp8e4 (exact 0/1) and kept RESIDENT in SBUF across both layers
    (~10.9 MB), removing the per-chunk S/ST streams entirely. Segment
    softmax-sum and message aggregation run as one fused PE matmul per tile
    with rhs [exp | exp*feat] (bf16 moving, fp8 stationary); e_dst expands via
    an ST x a_dst matmul. The softmax skips the max subtraction (scores are
    provably < ~4, exp is safe) and keeps the reference's +1e-16 epsilon.
  - Between layers, one AllGather exchanges the bf16 [h2 | a_src2] node table
    (the only cross-core traffic).
"""
import sys

sys.path.insert(0, "/opt/trn_rl_repo")

import ml_dtypes
import numpy as np

N_NODES = 10000
N_CORES = 8
NPC = N_NODES // N_CORES          # 1250
B_BLOCKS = 10
NPB = NPC // B_BLOCKS             # 125
TILE_E = 128
CHUNK = 8          # dma_gather ucode wedges above 1024 indices per call
N_QUEUES = 2       # alternate SWDGE queues -> gather desc-gen runs on two Q7 pairs
PAD_COL = 200.0
EPS = 1e-16
NEG_SLOPE = 0.2
F0, F1, F2, H1 = 128, 320, 64, 5
ROW1, ROW2 = 384, 128             # bf16 elements per table row


def _build_partition(edge_index):
    src = np.concatenate([edge_index[0].astype(np.int64),
                          np.arange(N_NODES, dtype=np.int64)])
    dst = np.concatenate([edge_index[1].astype(np.int64),
                          np.arange(N_NODES, dtype=np.int64)])
    core = dst // NPC
    block = (dst % NPC) // NPB
    col = dst % NPB

    cnt = np.zeros((N_CORES, B_BLOCKS), dtype=np.int64)
    np.add.at(cnt, (core, block), 1)
    T_b = np.ceil(cnt.max(axis=0) / TILE_E).astype(np.int64)
    tile_ofs = np.concatenate([[0], np.cumsum(T_b)])
    Ttot = int(tile_ofs[-1])
    Epad = Ttot * TILE_E

    src_sl = np.zeros((N_CORES, Epad), dtype=np.int64)
    col_sl = np.full((N_CORES, Epad), PAD_COL, dtype=np.float32)
    order = np.lexsort((dst, core * B_BLOCKS + block))
    s_src, s_core, s_block, s_col = src[order], core[order], block[order], col[order]
    idx = 0
    for c in range(N_CORES):
        for b in range(B_BLOCKS):
            n = int(cnt[c, b])
            base = int(tile_ofs[b]) * TILE_E
            sl = slice(idx, idx + n)
            assert np.all(s_core[sl] == c) and np.all(s_block[sl] == b)
            src_sl[c, base:base + n] = s_src[sl]
            col_sl[c, base:base + n] = s_col[sl]
            idx += n
    assert idx == len(src)
    return src_sl, col_sl, tile_ofs, Ttot, Epad


def _wrap_idx16(idx):
    a = idx.astype(np.int16).reshape(-1, 16).T
    return np.tile(a, (8, 1))


def _host_prep(inputs):
    x = np.asarray(inputs["x"], dtype=np.float32)
    W1 = np.asarray(inputs["W1"], dtype=np.float32)
    att_src1 = np.asarray(inputs["att_src1"], dtype=np.float32)
    att_dst1 = np.asarray(inputs["att_dst1"], dtype=np.float32)
    b1 = np.asarray(inputs["b1"], dtype=np.float32)
    W2 = np.asarray(inputs["W2"], dtype=np.float32)
    att_src2 = np.asarray(inputs["att_src2"], dtype=np.float32)
    att_dst2 = np.asarray(inputs["att_dst2"], dtype=np.float32)
    b2 = np.asarray(inputs["b2"], dtype=np.float32)
    ei = np.asarray(inputs["edge_index"])

    src_sl, col_sl, tile_ofs, Ttot, Epad = _build_partition(ei)

    A1 = np.zeros((F1, 2 * H1), dtype=np.float32)
    for h in range(H1):
        A1[64 * h:64 * h + 64, h] = att_src1[h]
        A1[64 * h:64 * h + 64, H1 + h] = att_dst1[h]
    W1A1 = (W1 @ A1).astype(np.float32)
    A2 = np.stack([att_src2[0], att_dst2[0]], axis=1).astype(np.float32)
    W2A2 = (W2 @ A2).astype(np.float32)

    xT = np.ascontiguousarray(x.T)
    shared = dict(
        xT16=xT.astype(ml_dtypes.bfloat16),
        W1c16=np.concatenate([W1, W1A1], axis=1).astype(ml_dtypes.bfloat16),
        W2=W2, W2A2=W2A2,
        ident=np.eye(128, dtype=np.float32),
        b1rep=np.broadcast_to(b1, (128, F1)).copy(),
        b2rep=np.broadcast_to(b2, (128, F2)).copy(),
    )
    d = np.arange(128, dtype=np.float32)
    per_core = []
    for c in range(N_CORES):
        colf = np.ascontiguousarray(col_sl[c].reshape(Ttot, TILE_E).T)
        S = (colf[:, :, None] == d[None, None, :])
        per_core.append(dict(
            src16=_wrap_idx16(src_sl[c]),
            S8=np.ascontiguousarray(S).astype(ml_dtypes.float8_e4m3),
            ST8=np.ascontiguousarray(np.transpose(S, (2, 1, 0))).astype(ml_dtypes.float8_e4m3),
            xTc16=np.ascontiguousarray(
                xT[:, c * NPC:(c + 1) * NPC]).astype(ml_dtypes.bfloat16),
        ))
    return shared, per_core, tile_ofs, Ttot, Epad


def _build_program(tile_ofs, Ttot, Epad):
    import concourse.bacc as bacc
    import concourse.mybir as mybir
    from concourse import tile

    dt = mybir.dt
    F32 = dt.float32
    BF16 = dt.bfloat16
    FP8 = dt.float8e4
    AF = mybir.ActivationFunctionType
    OP = mybir.AluOpType

    N = N_NODES
    B = B_BLOCKS
    tile_ofs = [int(v) for v in tile_ofs]
    block_of_tile = np.zeros(Ttot, dtype=np.int64)
    for b in range(B):
        block_of_tile[tile_ofs[b]:tile_ofs[b + 1]] = b
    n_node_tiles = (N + 127) // 128
    n_chunks = (Ttot + CHUNK - 1) // CHUNK

    nc = bacc.Bacc("TRN2", target_bir_lowering=False, debug=False,
                   num_devices=N_CORES, num_swdge_queues=N_QUEUES)

    xT_d = nc.dram_tensor("xT16", [F0, N], BF16, kind="ExternalInput")
    xTc_d = nc.dram_tensor("xTc16", [F0, NPC], BF16, kind="ExternalInput")
    W1c_d = nc.dram_tensor("W1c16", [F0, F1 + 2 * H1], BF16, kind="ExternalInput")
    W2_d = nc.dram_tensor("W2", [F1, F2], F32, kind="ExternalInput")
    W2A2_d = nc.dram_tensor("W2A2", [F1, 2], F32, kind="ExternalInput")
    ident_d = nc.dram_tensor("ident", [128, 128], F32, kind="ExternalInput")
    b1_d = nc.dram_tensor("b1rep", [128, F1], F32, kind="ExternalInput")
    b2_d = nc.dram_tensor("b2rep", [128, F2], F32, kind="ExternalInput")
    src16_d = nc.dram_tensor("src16", [128, Epad // 16], dt.int16, kind="ExternalInput")
    S_d = nc.dram_tensor("S8", [128, Ttot, 128], FP8, kind="ExternalInput")
    ST_d = nc.dram_tensor("ST8", [128, Ttot, 128], FP8, kind="ExternalInput")
    out_d = nc.dram_tensor("out", [NPC, F2], F32, kind="ExternalOutput")

    with tile.TileContext(nc) as tc:
        with (
            tc.tile_pool(name="dram", bufs=1, space="DRAM") as dram,
            tc.tile_pool(name="const", bufs=1) as cpool,
        ):
            h1tab = dram.tile([N, ROW1], BF16)
            ag_in = dram.tile([NPC, ROW2], BF16)
            h2tab = dram.tile([N, ROW2], BF16, addr_space="Shared")

            # resident indicator matrices (fp8, both layers); loaded on the
            # Activation engine's HWDGE queue so phase A keeps the sync queue
            S_res = cpool.tile([128, Ttot, 128], FP8)
            nc.scalar.dma_start(S_res[:], S_d[:])
            ST_res = cpool.tile([128, Ttot, 128], FP8)
            nc.scalar.dma_start(ST_res[:], ST_d[:])

            W1cs = cpool.tile([F0, F1 + 2 * H1], BF16)
            nc.sync.dma_start(W1cs[:], W1c_d[:])
            W2s = cpool.tile([128, 3, F2], F32)
            W2A2s = cpool.tile([128, 3, 2], F32)
            for k in range(3):
                w3 = min(128, F1 - 128 * k)
                nc.sync.dma_start(W2s[:w3, k, :], W2_d[128 * k:128 * k + w3, :])
                nc.sync.dma_start(W2A2s[:w3, k, :], W2A2_d[128 * k:128 * k + w3, :])
            idents = cpool.tile([128, 128], F32)
            nc.sync.dma_start(idents[:], ident_d[:])
            b1s = cpool.tile([128, F1], F32)
            nc.sync.dma_start(b1s[:], b1_d[:])
            b2s = cpool.tile([128, F2], F32)
            nc.sync.dma_start(b2s[:], b2_d[:])
            src16 = cpool.tile([128, Epad // 16], dt.int16)
            nc.sync.dma_start(src16[:], src16_d[:])
            adst1 = [cpool.tile([128, H1], BF16, name=f"adst1_{b}") for b in range(B)]
            adst2 = [cpool.tile([128, 1], BF16, name=f"adst2_{b}") for b in range(B)]

            # ---------------- phase A: h1/a1 node tables (replicated) ----
            with (
                tc.tile_pool(name="pA", bufs=4) as pA,
                tc.tile_pool(name="pAx", bufs=1) as pAx,
                tc.tile_pool(name="psA", bufs=2, space="PSUM") as psA,
            ):
                xTcs = pA.tile([F0, NPC], BF16, tag="xTc", bufs=1)
                nc.sync.dma_start(xTcs[:], xTc_d[:])
                for b in range(B):
                    pb = psA.tile([128, H1], F32, tag="pb")
                    nc.tensor.matmul(pb[:NPB, :], xTcs[:, NPB * b:NPB * (b + 1)],
                                     W1cs[:, F1 + H1:F1 + 2 * H1], start=True, stop=True)
                    nc.vector.tensor_copy(adst1[b][:NPB, :], pb[:NPB, :])
                PART = 13 * 128
                xparts = []
                for p in range(0, N, PART):
                    wp = min(PART, N - p)
                    xp = pAx.tile([F0, wp], BF16, name=f"xp{p}")
                    nc.sync.dma_start(xp[:], xT_d[:, p:p + wp])
                    xparts.append(xp)
                for nt in range(n_node_tiles):
                    w = min(128, N - 128 * nt)
                    pi, po = divmod(128 * nt, PART)
                    lhs = xparts[pi][:, po:po + w]
                    ph = psA.tile([128, F1 + 2 * H1], F32, tag="ph")
                    nc.tensor.matmul(ph[:w, :], lhs, W1cs[:], start=True, stop=True)
                    row = pA.tile([128, ROW1], BF16, tag="row")
                    nc.vector.memset(row[:w, F1 + H1:ROW1], 0.0)
                    nc.vector.tensor_copy(row[:w, 0:F1 + H1], ph[:w, 0:F1 + H1])
                    nc.sync.dma_start(h1tab[128 * nt:128 * nt + w, :], row[:w, :])

            # ---------------- shared edge-layer runner -------------------
            def edge_layer(tab, F, H, ROWT, adst, bias, out_sink):
                C = F // H
                with (
                    tc.tile_pool(name="gbuf", bufs=2) as gbuf,
                    tc.tile_pool(name="sbuf2", bufs=3) as sb2,
                    tc.tile_pool(name="ps_ed", bufs=2, space="PSUM") as ps_ed,
                    tc.tile_pool(name="ps_acc", bufs=2, space="PSUM") as ps_acc,
                    tc.tile_pool(name="ps_epi", bufs=2, space="PSUM") as ps_epi,
                    tc.tile_pool(name="epi", bufs=2) as epi,
                ):
                    usp = None
                    for ch in range(n_chunks):
                        t0 = ch * CHUNK
                        tn = min(CHUNK, Ttot - t0)
                        g = gbuf.tile([128, tn, ROWT], BF16, tag="g", bufs=6)
                        nc.gpsimd.dma_gather(
                            g[:], tab[:], src16[:, t0 * 8:t0 * 8 + tn * 8],
                            num_idxs=tn * 128, num_idxs_reg=tn * 128,
                            elem_size=ROWT, queue_num=ch % N_QUEUES)

                        edp = ps_ed.tile([128, tn, H], F32, tag="edp")
                        for tl in range(tn):
                            t = t0 + tl
                            b = int(block_of_tile[t])
                            nc.tensor.matmul(edp[:, tl, :], ST_res[:NPB, t, :],
                                             adst[b][:NPB, :], start=True, stop=True)
                        es = sb2.tile([128, tn, H], F32, tag="es", bufs=4)
                        nc.vector.tensor_tensor(
                            es[:], g[:, :, F:F + H], edp[:], OP.add)
                        nc.vector.scalar_tensor_tensor(
                            es[:], es[:], NEG_SLOPE, es[:], OP.mult, OP.max)
                        exhs = sb2.tile([128, tn, H + F], BF16, tag="exhs", bufs=4)
                        nc.scalar.activation(exhs[:, :, 0:H], es[:], AF.Exp)
                        g4 = g[:, :, 0:F].rearrange("p t (h c) -> p t h c", h=H)
                        hs4 = exhs[:, :, H:H + F].rearrange("p t (h c) -> p t h c", h=H)
                        ex4 = exhs[:, :, 0:H].unsqueeze(3).broadcast_to((128, tn, H, C))
                        nc.vector.tensor_tensor(hs4, g4, ex4, OP.mult)

                        for tl in range(tn):
                            t = t0 + tl
                            b = int(block_of_tile[t])
                            first = t == tile_ofs[b]
                            last = t == tile_ofs[b + 1] - 1
                            if first:
                                usp = ps_acc.tile([128, H + F], F32, tag="usp")
                            nc.tensor.matmul(usp[:NPB, :], S_res[:, t, 0:NPB],
                                             exhs[:, tl, :], start=first, stop=last)
                            if last:
                                rec = epi.tile([128, H], F32, tag="rec")
                                nc.vector.tensor_scalar_add(rec[:NPB, :], usp[:NPB, 0:H], EPS)
                                nc.vector.reciprocal(rec[:NPB, :], rec[:NPB, :])
                                o1 = epi.tile([128, F], F32, tag="o1")
                                for h in range(H):
                                    nc.vector.scalar_tensor_tensor(
                                        o1[:NPB, C * h:C * h + C],
                                        usp[:NPB, H + C * h:H + C * h + C],
                                        rec[:NPB, h:h + 1],
                                        bias[:NPB, C * h:C * h + C],
                                        OP.mult, OP.add)
                                o1r = epi.tile([128, F], F32, tag="o1r")
                                nc.scalar.activation(o1r[:NPB, :], o1[:NPB, :], AF.Relu)
                                out_sink(b, o1r, ps_epi, epi)

            # ---------------- layer 1 + inter-layer prep -----------------
            def sink1(b, o1r, ps_epi, epi):
                h1T = epi.tile([128, 3, NPB], F32, tag="h1T")
                for k in range(3):
                    w3 = min(128, F1 - 128 * k)
                    tp = ps_epi.tile([128, NPB], F32, tag="tp")
                    nc.tensor.transpose(tp[:w3, :], o1r[:NPB, 128 * k:128 * k + w3],
                                        idents[:NPB, :NPB])
                    nc.vector.tensor_copy(h1T[:w3, k, :], tp[:w3, :])
                h2ps = ps_epi.tile([128, F2 + 2], F32, tag="h2ps")
                for k in range(3):
                    w3 = min(128, F1 - 128 * k)
                    nc.tensor.matmul(h2ps[:NPB, 0:F2], h1T[:w3, k, :],
                                     W2s[:w3, k, :], start=(k == 0), stop=(k == 2))
                for k in range(3):
                    w3 = min(128, F1 - 128 * k)
                    nc.tensor.matmul(h2ps[:NPB, F2:F2 + 2], h1T[:w3, k, :],
                                     W2A2s[:w3, k, :], start=(k == 0), stop=(k == 2))
                agrow = epi.tile([128, ROW2], BF16, tag="agrow")
                nc.vector.memset(agrow[:NPB, F2 + 1:ROW2], 0.0)
                nc.vector.tensor_copy(agrow[:NPB, 0:F2 + 1], h2ps[:NPB, 0:F2 + 1])
                nc.sync.dma_start(ag_in[NPB * b:NPB * (b + 1), :], agrow[:NPB, :])
                nc.vector.tensor_copy(adst2[b][:NPB, :], h2ps[:NPB, F2 + 1:F2 + 2])

            edge_layer(h1tab, F1, H1, ROW1, adst1, b1s, sink1)

            nc.gpsimd.collective_compute(
                "AllGather", OP.bypass, replica_groups=[list(range(N_CORES))],
                ins=[ag_in.opt()], outs=[h2tab.opt()])

            def sink2(b, o2r, ps_epi, epi):
                nc.sync.dma_start(out_d[NPB * b:NPB * (b + 1), :], o2r[:NPB, 0:F2])

            edge_layer(h2tab, F2, 1, ROW2, adst2, b2s, sink2)

    nc.compile()
    return nc


def kernel(**inputs) -> np.ndarray:
    import time

    from concourse.bass_utils import run_bass_kernel_spmd

    shared, per_core, tile_ofs, Ttot, Epad = _host_prep(inputs)
    nc = _build_program(tile_ofs, Ttot, Epad)

    in_maps = []
    for c in range(N_CORES):
        m = dict(shared)
        m.update(per_core[c])
        in_maps.append(m)
    res = None
    for attempt in range(3):
        try:
            res = run_bass_kernel_spmd(nc, in_maps, list(range(N_CORES)))
            break
        except Exception:
            if attempt == 2:
                raise
            time.sleep(5)
    out = np.concatenate([res.results[c]["out"] for c in range(N_CORES)], axis=0)
    return np.ascontiguousarray(out.astype(np.float32))


# revision 18
# speedup vs baseline: 1.6625x; 1.0534x over previous
"""Trainium2 Bass kernel for the 2-layer GAT block (nn_GATblock_58282706206740).

Strategy (8 NeuronCores, SPMD):
  - Edges (incl. self-loops) are sharded by destination-node range: core c owns
    dst nodes [1250c, 1250(c+1)), split into 10 fixed blocks of 125 nodes.
    Per (core, block) edge lists are padded to a common per-block tile count
    (max over cores), so one program serves all cores with per-core data.
  - Per-node tables live in DRAM in bf16; per-edge rows arrive via gpsimd
    dma_gather (CHUNK tiles x 128 indices per call). The layer-1 table row
    packs [h1 | a_src1] as 384 bf16 (768B); the layer-2 table packs
    [h2 | a_src2] as 128 bf16 (256B).
  - Per-tile indicator matrices S [128e x 128d] / ST are precomputed on host
    in fp8e4 (exact 0/1) and kept RESIDENT in SBUF across both layers
    (~10.9 MB), removing the per-chunk S/ST streams entirely. Segment
    softmax-sum and message aggregation run as one fused PE matmul per tile
    with rhs [exp | exp*feat] (bf16 moving, fp8 stationary); e_dst expands via
    an ST x a_dst matmul. The softmax skips the max subtraction (scores are
    provably < ~4, exp is safe) and keeps the reference's +1e-16 epsilon.
  - Between layers, one AllGather exchanges the bf16 [h2 | a_src2] node table
    (the only cross-core traffic).
"""
import sys

sys.path.insert(0, "/opt/trn_rl_repo")

import ml_dtypes
import numpy as np

N_NODES = 10000
N_CORES = 8
NPC = N_NODES // N_CORES          # 1250
B_BLOCKS = 10
NPB = NPC // B_BLOCKS             # 125
TILE_E = 128
CHUNK = 8          # dma_gather ucode wedges above 1024 indices per call
N_QUEUES = 2       # alternate SWDGE queues -> gather desc-gen runs on two Q7 pairs
AG_HALF = 625      # rows per core per AllGather half (5 blocks)
PAD_COL = 200.0
EPS = 1e-16
NEG_SLOPE = 0.2
F0, F1, F2, H1 = 128, 320, 64, 5
ROW1, ROW2 = 384, 128             # bf16 elements per table row


def _build_partition(edge_index):
    src = np.concatenate([edge_index[0].astype(np.int64),
                          np.arange(N_NODES, dtype=np.int64)])
    dst = np.concatenate([edge_index[1].astype(np.int64),
                          np.arange(N_NODES, dtype=np.int64)])
    core = dst // NPC
    block = (dst % NPC) // NPB
    col = dst % NPB

    cnt = np.zeros((N_CORES, B_BLOCKS), dtype=np.int64)
    np.add.at(cnt, (core, block), 1)
    T_b = np.ceil(cnt.max(axis=0) / TILE_E).astype(np.int64)
    tile_ofs = np.concatenate([[0], np.cumsum(T_b)])
    Ttot = int(tile_ofs[-1])
    Epad = Ttot * TILE_E

    src_sl = np.zeros((N_CORES, Epad), dtype=np.int64)
    col_sl = np.full((N_CORES, Epad), PAD_COL, dtype=np.float32)
    order = np.lexsort((dst, core * B_BLOCKS + block))
    s_src, s_core, s_block, s_col = src[order], core[order], block[order], col[order]
    idx = 0
    for c in range(N_CORES):
        for b in range(B_BLOCKS):
            n = int(cnt[c, b])
            base = int(tile_ofs[b]) * TILE_E
            sl = slice(idx, idx + n)
            assert np.all(s_core[sl] == c) and np.all(s_block[sl] == b)
            src_sl[c, base:base + n] = s_src[sl]
            col_sl[c, base:base + n] = s_col[sl]
            idx += n
    assert idx == len(src)
    return src_sl, col_sl, tile_ofs, Ttot, Epad


def _wrap_idx16(idx):
    a = idx.astype(np.int16).reshape(-1, 16).T
    return np.tile(a, (8, 1))


def _host_prep(inputs):
    x = np.asarray(inputs["x"], dtype=np.float32)
    W1 = np.asarray(inputs["W1"], dtype=np.float32)
    att_src1 = np.asarray(inputs["att_src1"], dtype=np.float32)
    att_dst1 = np.asarray(inputs["att_dst1"], dtype=np.float32)
    b1 = np.asarray(inputs["b1"], dtype=np.float32)
    W2 = np.asarray(inputs["W2"], dtype=np.float32)
    att_src2 = np.asarray(inputs["att_src2"], dtype=np.float32)
    att_dst2 = np.asarray(inputs["att_dst2"], dtype=np.float32)
    b2 = np.asarray(inputs["b2"], dtype=np.float32)
    ei = np.asarray(inputs["edge_index"])

    src_sl, col_sl, tile_ofs, Ttot, Epad = _build_partition(ei)

    A1 = np.zeros((F1, 2 * H1), dtype=np.float32)
    for h in range(H1):
        A1[64 * h:64 * h + 64, h] = att_src1[h]
        A1[64 * h:64 * h + 64, H1 + h] = att_dst1[h]
    W1A1 = (W1 @ A1).astype(np.float32)
    A2 = np.stack([att_src2[0], att_dst2[0]], axis=1).astype(np.float32)
    W2A2 = (W2 @ A2).astype(np.float32)

    W2c = np.zeros((128, 3, F2 + 2), dtype=np.float32)
    for k in range(3):
        w3 = min(128, F1 - 128 * k)
        W2c[:w3, k, 0:F2] = W2[128 * k:128 * k + w3]
        W2c[:w3, k, F2:F2 + 2] = W2A2[128 * k:128 * k + w3]

    xT = np.ascontiguousarray(x.T)
    shared = dict(
        xT16=xT.astype(ml_dtypes.bfloat16),
        W1c16=np.concatenate([W1, W1A1], axis=1).astype(ml_dtypes.bfloat16),
        W2c=W2c,
        ident=np.eye(128, dtype=np.float32),
        b1rep=np.broadcast_to(b1, (128, F1)).copy(),
        b2rep=np.broadcast_to(b2, (128, F2)).copy(),
    )
    d = np.arange(128, dtype=np.float32)
    per_core = []
    for c in range(N_CORES):
        colf = np.ascontiguousarray(col_sl[c].reshape(Ttot, TILE_E).T)
        S = (colf[:, :, None] == d[None, None, :])
        # layer-2 table rows land in AllGather-half-major order:
        # node n -> half*5000 + (n//NPC)*AG_HALF + n%AG_HALF
        n_ = src_sl[c]
        half = (n_ % NPC) // AG_HALF
        l2row = half * (N_CORES * AG_HALF) + (n_ // NPC) * AG_HALF + n_ % AG_HALF
        per_core.append(dict(
            src16=_wrap_idx16(src_sl[c]),
            src16L2=_wrap_idx16(l2row),
            S8=np.ascontiguousarray(S).astype(ml_dtypes.float8_e4m3),
            ST8=np.ascontiguousarray(np.transpose(S, (2, 1, 0))).astype(ml_dtypes.float8_e4m3),
            xTc16=np.ascontiguousarray(
                xT[:, c * NPC:(c + 1) * NPC]).astype(ml_dtypes.bfloat16),
        ))
    return shared, per_core, tile_ofs, Ttot, Epad


def _build_program(tile_ofs, Ttot, Epad):
    import concourse.bacc as bacc
    import concourse.mybir as mybir
    from concourse import tile

    dt = mybir.dt
    F32 = dt.float32
    BF16 = dt.bfloat16
    FP8 = dt.float8e4
    AF = mybir.ActivationFunctionType
    OP = mybir.AluOpType

    N = N_NODES
    B = B_BLOCKS
    tile_ofs = [int(v) for v in tile_ofs]
    block_of_tile = np.zeros(Ttot, dtype=np.int64)
    for b in range(B):
        block_of_tile[tile_ofs[b]:tile_ofs[b + 1]] = b
    n_node_tiles = (N + 127) // 128
    n_chunks = (Ttot + CHUNK - 1) // CHUNK

    nc = bacc.Bacc("TRN2", target_bir_lowering=False, debug=False,
                   num_devices=N_CORES, num_swdge_queues=N_QUEUES)

    xT_d = nc.dram_tensor("xT16", [F0, N], BF16, kind="ExternalInput")
    xTc_d = nc.dram_tensor("xTc16", [F0, NPC], BF16, kind="ExternalInput")
    W1c_d = nc.dram_tensor("W1c16", [F0, F1 + 2 * H1], BF16, kind="ExternalInput")
    W2c_d = nc.dram_tensor("W2c", [128, 3, F2 + 2], F32, kind="ExternalInput")
    ident_d = nc.dram_tensor("ident", [128, 128], F32, kind="ExternalInput")
    b1_d = nc.dram_tensor("b1rep", [128, F1], F32, kind="ExternalInput")
    b2_d = nc.dram_tensor("b2rep", [128, F2], F32, kind="ExternalInput")
    src16_d = nc.dram_tensor("src16", [128, Epad // 16], dt.int16, kind="ExternalInput")
    src16L2_d = nc.dram_tensor("src16L2", [128, Epad // 16], dt.int16, kind="ExternalInput")
    S_d = nc.dram_tensor("S8", [128, Ttot, 128], FP8, kind="ExternalInput")
    ST_d = nc.dram_tensor("ST8", [128, Ttot, 128], FP8, kind="ExternalInput")
    out_d = nc.dram_tensor("out", [NPC, F2], F32, kind="ExternalOutput")

    with tile.TileContext(nc) as tc:
        with (
            tc.tile_pool(name="dram", bufs=1, space="DRAM") as dram,
            tc.tile_pool(name="const", bufs=1) as cpool,
        ):
            h1tab = dram.tile([N, ROW1], BF16)
            ag_inA = dram.tile([AG_HALF, ROW2], BF16)
            ag_inB = dram.tile([AG_HALF, ROW2], BF16)
            h2tabA = dram.tile([N_CORES * AG_HALF, ROW2], BF16, addr_space="Shared")
            h2tabB = dram.tile([N_CORES * AG_HALF, ROW2], BF16, addr_space="Shared")
            h2tab = dram.tile([N, ROW2], BF16)

            # phase-A-critical loads first (sync queue issues serialize at
            # ~0.7us each); big S/ST residents go on the Activation queue
            PART = 13 * 128
            xparts = []
            xp0 = cpool.tile([F0, PART], BF16, name="xp0")
            nc.sync.dma_start(xp0[:], xT_d[:, 0:PART])
            xparts.append(xp0)
            W1cs = cpool.tile([F0, F1 + 2 * H1], BF16)
            nc.sync.dma_start(W1cs[:], W1c_d[:])
            xTcs = cpool.tile([F0, NPC], BF16)
            nc.sync.dma_start(xTcs[:], xTc_d[:])
            src16 = cpool.tile([128, Epad // 16], dt.int16)
            nc.sync.dma_start(src16[:], src16_d[:])
            for p in range(PART, N, PART):
                wp = min(PART, N - p)
                xp = cpool.tile([F0, wp], name=f"xp{p}", dtype=BF16)
                nc.sync.dma_start(xp[:], xT_d[:, p:p + wp])
                xparts.append(xp)

            S_res = cpool.tile([128, Ttot, 128], FP8)
            nc.scalar.dma_start(S_res[:], S_d[:])
            ST_res = cpool.tile([128, Ttot, 128], FP8)
            nc.scalar.dma_start(ST_res[:], ST_d[:])
            src16L2 = cpool.tile([128, Epad // 16], dt.int16)
            nc.scalar.dma_start(src16L2[:], src16L2_d[:])
            W2s = cpool.tile([128, 3, F2 + 2], F32)
            nc.scalar.dma_start(W2s[:], W2c_d[:])
            idents = cpool.tile([128, 128], F32)
            nc.scalar.dma_start(idents[:], ident_d[:])
            b1s = cpool.tile([128, F1], F32)
            nc.scalar.dma_start(b1s[:], b1_d[:])
            b2s = cpool.tile([128, F2], F32)
            nc.scalar.dma_start(b2s[:], b2_d[:])
            adst1 = [cpool.tile([128, H1], BF16, name=f"adst1_{b}") for b in range(B)]
            adst2 = [cpool.tile([128, 1], BF16, name=f"adst2_{b}") for b in range(B)]

            # ---------------- phase A: h1/a1 node tables (replicated) ----
            WGRP = 8                       # node-tiles per h1tab write
            with (
                tc.tile_pool(name="pA", bufs=2) as pA,
                tc.tile_pool(name="psA", bufs=2, space="PSUM") as psA,
            ):
                for b in range(B):
                    pb = psA.tile([128, H1], F32, tag="pb")
                    nc.tensor.matmul(pb[:NPB, :], xTcs[:, NPB * b:NPB * (b + 1)],
                                     W1cs[:, F1 + H1:F1 + 2 * H1], start=True, stop=True)
                    nc.vector.tensor_copy(adst1[b][:NPB, :], pb[:NPB, :])
                full_tiles = N // 128
                rowblk = None
                for nt in range(full_tiles):
                    j = nt % WGRP
                    if j == 0:
                        gn = min(WGRP, full_tiles - nt)
                        rowblk = pA.tile([128, gn, ROW1], BF16, tag="rowblk")
                        nc.vector.memset(rowblk[:, :, F1 + H1:ROW1], 0.0)
                    pi, po = divmod(128 * nt, PART)
                    lhs = xparts[pi][:, po:po + 128]
                    ph = psA.tile([128, F1 + 2 * H1], F32, tag="ph")
                    nc.tensor.matmul(ph[:], lhs, W1cs[:], start=True, stop=True)
                    nc.vector.tensor_copy(rowblk[:, j, 0:F1 + H1], ph[:, 0:F1 + H1])
                    if j == gn - 1:
                        nt0 = nt - j
                        dst = h1tab[nt0 * 128:(nt0 + gn) * 128, :].rearrange(
                            "(t p) f -> p t f", p=128)
                        nc.sync.dma_start(dst, rowblk[:, :gn, :])
                if N % 128:
                    w = N % 128
                    nt = full_tiles
                    pi, po = divmod(128 * nt, PART)
                    lhs = xparts[pi][:, po:po + w]
                    ph = psA.tile([128, F1 + 2 * H1], F32, tag="ph")
                    nc.tensor.matmul(ph[:w, :], lhs, W1cs[:], start=True, stop=True)
                    row = pA.tile([128, ROW1], BF16, tag="rowtail")
                    nc.vector.memset(row[:w, F1 + H1:ROW1], 0.0)
                    nc.vector.tensor_copy(row[:w, 0:F1 + H1], ph[:w, 0:F1 + H1])
                    nc.sync.dma_start(h1tab[128 * nt:128 * nt + w, :], row[:w, :])

            # ---------------- shared edge-layer runner -------------------
            def edge_layer(tab, F, H, ROWT, adst, bias, out_sink, idxs,
                           mid_hook=None):
                C = F // H
                with (
                    tc.tile_pool(name="gbuf", bufs=2) as gbuf,
                    tc.tile_pool(name="sbuf2", bufs=3) as sb2,
                    tc.tile_pool(name="ps_ed", bufs=2, space="PSUM") as ps_ed,
                    tc.tile_pool(name="ps_acc", bufs=2, space="PSUM") as ps_acc,
                    tc.tile_pool(name="ps_epi", bufs=2, space="PSUM") as ps_epi,
                    tc.tile_pool(name="epi", bufs=2) as epi,
                ):
                    usp = None
                    for ch in range(n_chunks):
                        if mid_hook is not None:
                            mid_hook(ch)
                        t0 = ch * CHUNK
                        tn = min(CHUNK, Ttot - t0)
                        g = gbuf.tile([128, tn, ROWT], BF16, tag="g", bufs=6)
                        nc.gpsimd.dma_gather(
                            g[:], tab[:], idxs[:, t0 * 8:t0 * 8 + tn * 8],
                            num_idxs=tn * 128, num_idxs_reg=tn * 128,
                            elem_size=ROWT, queue_num=ch % N_QUEUES)

                        edp = ps_ed.tile([128, tn, H], F32, tag="edp")
                        for tl in range(tn):
                            t = t0 + tl
                            b = int(block_of_tile[t])
                            nc.tensor.matmul(edp[:, tl, :], ST_res[:NPB, t, :],
                                             adst[b][:NPB, :], start=True, stop=True)
                        es = sb2.tile([128, tn, H], F32, tag="es", bufs=4)
                        nc.vector.tensor_tensor(
                            es[:], g[:, :, F:F + H], edp[:], OP.add)
                        nc.vector.scalar_tensor_tensor(
                            es[:], es[:], NEG_SLOPE, es[:], OP.mult, OP.max)
                        exhs = sb2.tile([128, tn, H + F], BF16, tag="exhs", bufs=4)
                        nc.scalar.activation(exhs[:, :, 0:H], es[:], AF.Exp)
                        g4 = g[:, :, 0:F].rearrange("p t (h c) -> p t h c", h=H)
                        hs4 = exhs[:, :, H:H + F].rearrange("p t (h c) -> p t h c", h=H)
                        ex4 = exhs[:, :, 0:H].unsqueeze(3).broadcast_to((128, tn, H, C))
                        nc.vector.tensor_tensor(hs4, g4, ex4, OP.mult)

                        for tl in range(tn):
                            t = t0 + tl
                            b = int(block_of_tile[t])
                            first = t == tile_ofs[b]
                            last = t == tile_ofs[b + 1] - 1
                            if first:
                                usp = ps_acc.tile([128, H + F], F32, tag="usp")
                            nc.tensor.matmul(usp[:NPB, :], S_res[:, t, 0:NPB],
                                             exhs[:, tl, :], start=first, stop=last)
                            if last:
                                rec = epi.tile([128, H], F32, tag="rec")
                                nc.vector.tensor_scalar_add(rec[:NPB, :], usp[:NPB, 0:H], EPS)
                                nc.vector.reciprocal(rec[:NPB, :], rec[:NPB, :])
                                o1 = epi.tile([128, F], F32, tag="o1")
                                for h in range(H):
                                    nc.vector.scalar_tensor_tensor(
                                        o1[:NPB, C * h:C * h + C],
                                        usp[:NPB, H + C * h:H + C * h + C],
                                        rec[:NPB, h:h + 1],
                                        bias[:NPB, C * h:C * h + C],
                                        OP.mult, OP.add)
                                o1r = epi.tile([128, F], F32, tag="o1r")
                                nc.scalar.activation(o1r[:NPB, :], o1[:NPB, :], AF.Relu)
                                out_sink(b, o1r, ps_epi, epi)

            # ---------------- layer 1 + inter-layer prep -----------------
            def sink1(b, o1r, ps_epi, epi):
                h1T = epi.tile([128, 3, NPB], F32, tag="h1T")
                for k in range(3):
                    w3 = min(128, F1 - 128 * k)
                    tp = ps_epi.tile([128, NPB], F32, tag="tp")
                    nc.tensor.transpose(tp[:w3, :], o1r[:NPB, 128 * k:128 * k + w3],
                                        idents[:NPB, :NPB])
                    nc.vector.tensor_copy(h1T[:w3, k, :], tp[:w3, :])
                h2ps = ps_epi.tile([128, F2 + 2], F32, tag="h2ps")
                for k in range(3):
                    w3 = min(128, F1 - 128 * k)
                    nc.tensor.matmul(h2ps[:NPB, :], h1T[:w3, k, :],
                                     W2s[:w3, k, :], start=(k == 0), stop=(k == 2))
                agrow = epi.tile([128, ROW2], BF16, tag="agrow")
                nc.vector.memset(agrow[:NPB, F2 + 1:ROW2], 0.0)
                nc.vector.tensor_copy(agrow[:NPB, 0:F2 + 1], h2ps[:NPB, 0:F2 + 1])
                ag_dst = ag_inA if b < 5 else ag_inB
                r0 = (b % 5) * NPB
                nc.sync.dma_start(ag_dst[r0:r0 + NPB, :], agrow[:NPB, :])
                nc.vector.tensor_copy(adst2[b][:NPB, :], h2ps[:NPB, F2 + 1:F2 + 2])

            # fire the first AllGather half once blocks 0-4 have drained
            # through the pipeline (a few chunks after block 4's last tile)
            trig_ch = min(n_chunks - 1, (tile_ofs[5] - 1) // CHUNK + 3)
            emitted = [False]

            def mid_hook(ch):
                if ch == trig_ch and not emitted[0]:
                    emitted[0] = True
                    nc.gpsimd.collective_compute(
                        "AllGather", OP.bypass,
                        replica_groups=[list(range(N_CORES))],
                        ins=[ag_inA.opt()], outs=[h2tabA.opt()])
                    nc.sync.dma_start(h2tab[0:N_CORES * AG_HALF, :], h2tabA[:])

            edge_layer(h1tab, F1, H1, ROW1, adst1, b1s, sink1, src16, mid_hook)

            nc.gpsimd.collective_compute(
                "AllGather", OP.bypass, replica_groups=[list(range(N_CORES))],
                ins=[ag_inB.opt()], outs=[h2tabB.opt()])
            nc.sync.dma_start(h2tab[N_CORES * AG_HALF:2 * N_CORES * AG_HALF, :],
                              h2tabB[:])

            def sink2(b, o2r, ps_epi, epi):
                nc.sync.dma_start(out_d[NPB * b:NPB * (b + 1), :], o2r[:NPB, 0:F2])

            edge_layer(h2tab, F2, 1, ROW2, adst2, b2s, sink2, src16L2)

    nc.compile()
    return nc


def kernel(**inputs) -> np.ndarray:
    import time

    from concourse.bass_utils import run_bass_kernel_spmd

    shared, per_core, tile_ofs, Ttot, Epad = _host_prep(inputs)
    nc = _build_program(tile_ofs, Ttot, Epad)

    in_maps = []
    for c in range(N_CORES):
        m = dict(shared)
        m.update(per_core[c])
        in_maps.append(m)
    res = None
    for attempt in range(3):
        try:
            res = run_bass_kernel_spmd(nc, in_maps, list(range(N_CORES)))
            break
        except Exception:
            if attempt == 2:
                raise
            time.sleep(5)
    out = np.concatenate([res.results[c]["out"] for c in range(N_CORES)], axis=0)
    return np.ascontiguousarray(out.astype(np.float32))


# revision 21
# speedup vs baseline: 1.6930x; 1.0183x over previous
"""Trainium2 Bass kernel for the 2-layer GAT block (nn_GATblock_58282706206740).

Strategy (8 NeuronCores, SPMD):
  - Edges (incl. self-loops) are sharded by destination-node range: core c owns
    dst nodes [1250c, 1250(c+1)), split into 10 fixed blocks of 125 nodes.
    Per (core, block) edge lists are padded to a common per-block tile count
    (max over cores), so one program serves all cores with per-core data.
  - Per-node tables live in DRAM in bf16; per-edge rows arrive via gpsimd
    dma_gather (CHUNK tiles x 128 indices per call). The layer-1 table row
    packs [h1 | a_src1] as 384 bf16 (768B); the layer-2 table packs
    [h2 | a_src2] as 128 bf16 (256B).
  - Per-tile indicator matrices S [128e x 128d] / ST are precomputed on host
    in fp8e4 (exact 0/1) and kept RESIDENT in SBUF across both layers
    (~10.9 MB), removing the per-chunk S/ST streams entirely. Segment
    softmax-sum and message aggregation run as one fused PE matmul per tile
    with rhs [exp | exp*feat] (bf16 moving, fp8 stationary); e_dst expands via
    an ST x a_dst matmul. The softmax skips the max subtraction (scores are
    provably < ~4, exp is safe) and keeps the reference's +1e-16 epsilon.
  - Between layers, one AllGather exchanges the bf16 [h2 | a_src2] node table
    (the only cross-core traffic).
"""
import sys

sys.path.insert(0, "/opt/trn_rl_repo")

import ml_dtypes
import numpy as np

N_NODES = 10000
N_CORES = 8
NPC = N_NODES // N_CORES          # 1250
B_BLOCKS = 10
NPB = NPC // B_BLOCKS             # 125
TILE_E = 128
CHUNK = 8          # dma_gather ucode wedges above 1024 indices per call
N_QUEUES = 2       # alternate SWDGE queues -> gather desc-gen runs on two Q7 pairs
AG_HALF = 625      # rows per core per AllGather half (5 blocks)
PAD_COL = 200.0
EPS = 1e-16
NEG_SLOPE = 0.2
F0, F1, F2, H1 = 128, 320, 64, 5
ROW1, ROW2 = 384, 128             # bf16 elements per table row


def _build_partition(edge_index):
    src = np.concatenate([edge_index[0].astype(np.int64),
                          np.arange(N_NODES, dtype=np.int64)])
    dst = np.concatenate([edge_index[1].astype(np.int64),
                          np.arange(N_NODES, dtype=np.int64)])
    core = dst // NPC
    block = (dst % NPC) // NPB
    col = dst % NPB

    cnt = np.zeros((N_CORES, B_BLOCKS), dtype=np.int64)
    np.add.at(cnt, (core, block), 1)
    T_b = np.ceil(cnt.max(axis=0) / TILE_E).astype(np.int64)
    tile_ofs = np.concatenate([[0], np.cumsum(T_b)])
    Ttot = int(tile_ofs[-1])
    Epad = Ttot * TILE_E

    src_sl = np.zeros((N_CORES, Epad), dtype=np.int64)
    col_sl = np.full((N_CORES, Epad), PAD_COL, dtype=np.float32)
    order = np.lexsort((dst, core * B_BLOCKS + block))
    s_src, s_core, s_block, s_col = src[order], core[order], block[order], col[order]
    idx = 0
    for c in range(N_CORES):
        for b in range(B_BLOCKS):
            n = int(cnt[c, b])
            base = int(tile_ofs[b]) * TILE_E
            sl = slice(idx, idx + n)
            assert np.all(s_core[sl] == c) and np.all(s_block[sl] == b)
            src_sl[c, base:base + n] = s_src[sl]
            col_sl[c, base:base + n] = s_col[sl]
            idx += n
    assert idx == len(src)
    return src_sl, col_sl, tile_ofs, Ttot, Epad


def _wrap_idx16(idx):
    a = idx.astype(np.int16).reshape(-1, 16).T
    return np.tile(a, (8, 1))


def _host_prep(inputs):
    x = np.asarray(inputs["x"], dtype=np.float32)
    W1 = np.asarray(inputs["W1"], dtype=np.float32)
    att_src1 = np.asarray(inputs["att_src1"], dtype=np.float32)
    att_dst1 = np.asarray(inputs["att_dst1"], dtype=np.float32)
    b1 = np.asarray(inputs["b1"], dtype=np.float32)
    W2 = np.asarray(inputs["W2"], dtype=np.float32)
    att_src2 = np.asarray(inputs["att_src2"], dtype=np.float32)
    att_dst2 = np.asarray(inputs["att_dst2"], dtype=np.float32)
    b2 = np.asarray(inputs["b2"], dtype=np.float32)
    ei = np.asarray(inputs["edge_index"])

    src_sl, col_sl, tile_ofs, Ttot, Epad = _build_partition(ei)

    A1 = np.zeros((F1, 2 * H1), dtype=np.float32)
    for h in range(H1):
        A1[64 * h:64 * h + 64, h] = att_src1[h]
        A1[64 * h:64 * h + 64, H1 + h] = att_dst1[h]
    W1A1 = (W1 @ A1).astype(np.float32)
    A2 = np.stack([att_src2[0], att_dst2[0]], axis=1).astype(np.float32)
    W2A2 = (W2 @ A2).astype(np.float32)

    W2c = np.zeros((128, 3, F2 + 2), dtype=np.float32)
    for k in range(3):
        w3 = min(128, F1 - 128 * k)
        W2c[:w3, k, 0:F2] = W2[128 * k:128 * k + w3]
        W2c[:w3, k, F2:F2 + 2] = W2A2[128 * k:128 * k + w3]

    xT = np.ascontiguousarray(x.T)
    shared = dict(
        xT16=xT.astype(ml_dtypes.bfloat16),
        W1c16=np.concatenate([W1, W1A1], axis=1).astype(ml_dtypes.bfloat16),
        W2c=W2c,
        ident=np.eye(128, dtype=np.float32),
        b1rep=np.broadcast_to(b1, (128, F1)).copy(),
        b2rep=np.broadcast_to(b2, (128, F2)).copy(),
    )
    d = np.arange(128, dtype=np.float32)
    per_core = []
    for c in range(N_CORES):
        colf = np.ascontiguousarray(col_sl[c].reshape(Ttot, TILE_E).T)
        S = (colf[:, :, None] == d[None, None, :])
        # layer-2 table rows land in AllGather-half-major order:
        # node n -> half*5000 + (n//NPC)*AG_HALF + n%AG_HALF
        n_ = src_sl[c]
        half = (n_ % NPC) // AG_HALF
        l2row = half * (N_CORES * AG_HALF) + (n_ // NPC) * AG_HALF + n_ % AG_HALF
        per_core.append(dict(
            src16=_wrap_idx16(src_sl[c]),
            src16L2=_wrap_idx16(l2row),
            S8=np.ascontiguousarray(S).astype(ml_dtypes.float8_e4m3),
            ST8=np.ascontiguousarray(np.transpose(S, (2, 1, 0))).astype(ml_dtypes.float8_e4m3),
            xTc16=np.ascontiguousarray(
                xT[:, c * NPC:(c + 1) * NPC]).astype(ml_dtypes.bfloat16),
        ))
    return shared, per_core, tile_ofs, Ttot, Epad


def _build_program(tile_ofs, Ttot, Epad):
    import concourse.bacc as bacc
    import concourse.mybir as mybir
    from concourse import tile

    dt = mybir.dt
    F32 = dt.float32
    BF16 = dt.bfloat16
    FP8 = dt.float8e4
    AF = mybir.ActivationFunctionType
    OP = mybir.AluOpType

    N = N_NODES
    B = B_BLOCKS
    tile_ofs = [int(v) for v in tile_ofs]
    block_of_tile = np.zeros(Ttot, dtype=np.int64)
    for b in range(B):
        block_of_tile[tile_ofs[b]:tile_ofs[b + 1]] = b
    n_node_tiles = (N + 127) // 128
    n_chunks = (Ttot + CHUNK - 1) // CHUNK

    nc = bacc.Bacc("TRN2", target_bir_lowering=False, debug=False,
                   num_devices=N_CORES, num_swdge_queues=N_QUEUES)

    xT_d = nc.dram_tensor("xT16", [F0, N], BF16, kind="ExternalInput")
    xTc_d = nc.dram_tensor("xTc16", [F0, NPC], BF16, kind="ExternalInput")
    W1c_d = nc.dram_tensor("W1c16", [F0, F1 + 2 * H1], BF16, kind="ExternalInput")
    W2c_d = nc.dram_tensor("W2c", [128, 3, F2 + 2], F32, kind="ExternalInput")
    ident_d = nc.dram_tensor("ident", [128, 128], F32, kind="ExternalInput")
    b1_d = nc.dram_tensor("b1rep", [128, F1], F32, kind="ExternalInput")
    b2_d = nc.dram_tensor("b2rep", [128, F2], F32, kind="ExternalInput")
    src16_d = nc.dram_tensor("src16", [128, Epad // 16], dt.int16, kind="ExternalInput")
    src16L2_d = nc.dram_tensor("src16L2", [128, Epad // 16], dt.int16, kind="ExternalInput")
    S_d = nc.dram_tensor("S8", [128, Ttot, 128], FP8, kind="ExternalInput")
    ST_d = nc.dram_tensor("ST8", [128, Ttot, 128], FP8, kind="ExternalInput")
    out_d = nc.dram_tensor("out", [NPC, F2], F32, kind="ExternalOutput")

    with tile.TileContext(nc) as tc:
        with (
            tc.tile_pool(name="dram", bufs=1, space="DRAM") as dram,
            tc.tile_pool(name="const", bufs=1) as cpool,
        ):
            h1tab = dram.tile([N, ROW1], BF16)
            ag_inA = dram.tile([AG_HALF, ROW2], BF16)
            ag_inB = dram.tile([AG_HALF, ROW2], BF16)
            h2tabA = dram.tile([N_CORES * AG_HALF, ROW2], BF16, addr_space="Shared")
            h2tabB = dram.tile([N_CORES * AG_HALF, ROW2], BF16, addr_space="Shared")
            h2tab = dram.tile([N, ROW2], BF16)

            # phase-A-critical loads first (sync queue issues serialize at
            # ~0.7us each); big S/ST residents go on the Activation queue
            PART = 13 * 128
            xparts = []
            xp0 = cpool.tile([F0, PART], BF16, name="xp0")
            nc.sync.dma_start(xp0[:], xT_d[:, 0:PART])
            xparts.append(xp0)
            W1cs = cpool.tile([F0, F1 + 2 * H1], BF16)
            nc.sync.dma_start(W1cs[:], W1c_d[:])
            src16 = cpool.tile([128, Epad // 16], dt.int16)
            nc.sync.dma_start(src16[:], src16_d[:])
            for p in range(PART, N, PART):
                wp = min(PART, N - p)
                xp = cpool.tile([F0, wp], name=f"xp{p}", dtype=BF16)
                nc.sync.dma_start(xp[:], xT_d[:, p:p + wp])
                xparts.append(xp)
            xTcs = cpool.tile([F0, NPC], BF16)
            nc.sync.dma_start(xTcs[:], xTc_d[:])

            S_res = cpool.tile([128, Ttot, 128], FP8)
            nc.scalar.dma_start(S_res[:], S_d[:])
            ST_res = cpool.tile([128, Ttot, 128], FP8)
            nc.scalar.dma_start(ST_res[:], ST_d[:])
            src16L2 = cpool.tile([128, Epad // 16], dt.int16)
            nc.scalar.dma_start(src16L2[:], src16L2_d[:])
            W2s = cpool.tile([128, 3, F2 + 2], F32)
            nc.scalar.dma_start(W2s[:], W2c_d[:])
            idents = cpool.tile([128, 128], F32)
            nc.scalar.dma_start(idents[:], ident_d[:])
            b1s = cpool.tile([128, F1], F32)
            nc.scalar.dma_start(b1s[:], b1_d[:])
            b2s = cpool.tile([128, F2], F32)
            nc.scalar.dma_start(b2s[:], b2_d[:])
            adst1 = [cpool.tile([128, H1], BF16, name=f"adst1_{b}") for b in range(B)]
            adst2 = [cpool.tile([128, 1], BF16, name=f"adst2_{b}") for b in range(B)]

            # ---------------- phase A: h1/a1 node tables (replicated) ----
            WGRP = 8                       # node-tiles per h1tab write
            with (
                tc.tile_pool(name="pA", bufs=3) as pA,
                tc.tile_pool(name="psA", bufs=2, space="PSUM") as psA,
            ):
                full_tiles = N // 128
                rowblk = None
                for nt in range(full_tiles):
                    j = nt % WGRP
                    if j == 0:
                        gn = min(WGRP, full_tiles - nt)
                        rowblk = pA.tile([128, gn, ROW1], BF16, tag="rowblk")
                        nc.vector.memset(rowblk[:, :, F1 + H1:ROW1], 0.0)
                    pi, po = divmod(128 * nt, PART)
                    lhs = xparts[pi][:, po:po + 128]
                    ph = psA.tile([128, F1 + 2 * H1], F32, tag="ph", bufs=4)
                    nc.tensor.matmul(ph[:], lhs, W1cs[:], start=True, stop=True)
                    # alternate the PSUM->bf16 cast between DVE and ACT so
                    # neither engine gates the matmul pipeline
                    if nt % 2 == 0:
                        nc.vector.tensor_copy(rowblk[:, j, 0:F1 + H1], ph[:, 0:F1 + H1])
                    else:
                        nc.scalar.activation(rowblk[:, j, 0:F1 + H1], ph[:, 0:F1 + H1],
                                             AF.Copy)
                    if j == gn - 1:
                        nt0 = nt - j
                        dst = h1tab[nt0 * 128:(nt0 + gn) * 128, :].rearrange(
                            "(t p) f -> p t f", p=128)
                        nc.sync.dma_start(dst, rowblk[:, :gn, :])
                if N % 128:
                    w = N % 128
                    nt = full_tiles
                    pi, po = divmod(128 * nt, PART)
                    lhs = xparts[pi][:, po:po + w]
                    ph = psA.tile([128, F1 + 2 * H1], F32, tag="ph", bufs=4)
                    nc.tensor.matmul(ph[:w, :], lhs, W1cs[:], start=True, stop=True)
                    row = pA.tile([128, ROW1], BF16, tag="rowtail")
                    nc.vector.memset(row[:w, F1 + H1:ROW1], 0.0)
                    nc.vector.tensor_copy(row[:w, 0:F1 + H1], ph[:w, 0:F1 + H1])
                    nc.sync.dma_start(h1tab[128 * nt:128 * nt + w, :], row[:w, :])
                # a_dst1 blocks last: only needed once layer-1 chunks start
                for b in range(B):
                    pb = psA.tile([128, H1], F32, tag="pb")
                    nc.tensor.matmul(pb[:NPB, :], xTcs[:, NPB * b:NPB * (b + 1)],
                                     W1cs[:, F1 + H1:F1 + 2 * H1], start=True, stop=True)
                    nc.vector.tensor_copy(adst1[b][:NPB, :], pb[:NPB, :])

            # ---------------- shared edge-layer runner -------------------
            def edge_layer(tab, F, H, ROWT, adst, bias, out_sink, idxs,
                           mid_hook=None):
                C = F // H
                with (
                    tc.tile_pool(name="gbuf", bufs=2) as gbuf,
                    tc.tile_pool(name="sbuf2", bufs=3) as sb2,
                    tc.tile_pool(name="ps_ed", bufs=2, space="PSUM") as ps_ed,
                    tc.tile_pool(name="ps_acc", bufs=2, space="PSUM") as ps_acc,
                    tc.tile_pool(name="ps_epi", bufs=2, space="PSUM") as ps_epi,
                    tc.tile_pool(name="epi", bufs=2) as epi,
                ):
                    usp = None
                    for ch in range(n_chunks):
                        if mid_hook is not None:
                            mid_hook(ch)
                        t0 = ch * CHUNK
                        tn = min(CHUNK, Ttot - t0)
                        g = gbuf.tile([128, tn, ROWT], BF16, tag="g", bufs=6)
                        nc.gpsimd.dma_gather(
                            g[:], tab[:], idxs[:, t0 * 8:t0 * 8 + tn * 8],
                            num_idxs=tn * 128, num_idxs_reg=tn * 128,
                            elem_size=ROWT, queue_num=ch % N_QUEUES)

                        edp = ps_ed.tile([128, tn, H], F32, tag="edp")
                        for tl in range(tn):
                            t = t0 + tl
                            b = int(block_of_tile[t])
                            nc.tensor.matmul(edp[:, tl, :], ST_res[:NPB, t, :],
                                             adst[b][:NPB, :], start=True, stop=True)
                        es = sb2.tile([128, tn, H], F32, tag="es", bufs=4)
                        nc.vector.tensor_tensor(
                            es[:], g[:, :, F:F + H], edp[:], OP.add)
                        nc.vector.scalar_tensor_tensor(
                            es[:], es[:], NEG_SLOPE, es[:], OP.mult, OP.max)
                        exhs = sb2.tile([128, tn, H + F], BF16, tag="exhs", bufs=4)
                        nc.scalar.activation(exhs[:, :, 0:H], es[:], AF.Exp)
                        g4 = g[:, :, 0:F].rearrange("p t (h c) -> p t h c", h=H)
                        hs4 = exhs[:, :, H:H + F].rearrange("p t (h c) -> p t h c", h=H)
                        ex4 = exhs[:, :, 0:H].unsqueeze(3).broadcast_to((128, tn, H, C))
                        nc.vector.tensor_tensor(hs4, g4, ex4, OP.mult)

                        for tl in range(tn):
                            t = t0 + tl
                            b = int(block_of_tile[t])
                            first = t == tile_ofs[b]
                            last = t == tile_ofs[b + 1] - 1
                            if first:
                                usp = ps_acc.tile([128, H + F], F32, tag="usp")
                            nc.tensor.matmul(usp[:NPB, :], S_res[:, t, 0:NPB],
                                             exhs[:, tl, :], start=first, stop=last)
                            if last:
                                rec = epi.tile([128, H], F32, tag="rec")
                                nc.vector.tensor_scalar_add(rec[:NPB, :], usp[:NPB, 0:H], EPS)
                                nc.vector.reciprocal(rec[:NPB, :], rec[:NPB, :])
                                o1 = epi.tile([128, F], F32, tag="o1")
                                for h in range(H):
                                    nc.vector.scalar_tensor_tensor(
                                        o1[:NPB, C * h:C * h + C],
                                        usp[:NPB, H + C * h:H + C * h + C],
                                        rec[:NPB, h:h + 1],
                                        bias[:NPB, C * h:C * h + C],
                                        OP.mult, OP.add)
                                o1r = epi.tile([128, F], F32, tag="o1r")
                                nc.scalar.activation(o1r[:NPB, :], o1[:NPB, :], AF.Relu)
                                out_sink(b, o1r, ps_epi, epi)

            # ---------------- layer 1 + inter-layer prep -----------------
            def sink1(b, o1r, ps_epi, epi):
                h1T = epi.tile([128, 3, NPB], F32, tag="h1T")
                for k in range(3):
                    w3 = min(128, F1 - 128 * k)
                    tp = ps_epi.tile([128, NPB], F32, tag="tp")
                    nc.tensor.transpose(tp[:w3, :], o1r[:NPB, 128 * k:128 * k + w3],
                                        idents[:NPB, :NPB])
                    nc.vector.tensor_copy(h1T[:w3, k, :], tp[:w3, :])
                h2ps = ps_epi.tile([128, F2 + 2], F32, tag="h2ps")
                for k in range(3):
                    w3 = min(128, F1 - 128 * k)
                    nc.tensor.matmul(h2ps[:NPB, :], h1T[:w3, k, :],
                                     W2s[:w3, k, :], start=(k == 0), stop=(k == 2))
                agrow = epi.tile([128, ROW2], BF16, tag="agrow")
                nc.vector.memset(agrow[:NPB, F2 + 1:ROW2], 0.0)
                nc.vector.tensor_copy(agrow[:NPB, 0:F2 + 1], h2ps[:NPB, 0:F2 + 1])
                ag_dst = ag_inA if b < 5 else ag_inB
                r0 = (b % 5) * NPB
                nc.sync.dma_start(ag_dst[r0:r0 + NPB, :], agrow[:NPB, :])
                nc.vector.tensor_copy(adst2[b][:NPB, :], h2ps[:NPB, F2 + 1:F2 + 2])

            # fire the first AllGather half once blocks 0-4 have drained
            # through the pipeline (a few chunks after block 4's last tile)
            trig_ch = min(n_chunks - 1, (tile_ofs[5] - 1) // CHUNK + 6)
            emitted = [False]

            def mid_hook(ch):
                if ch == trig_ch and not emitted[0]:
                    emitted[0] = True
                    nc.gpsimd.collective_compute(
                        "AllGather", OP.bypass,
                        replica_groups=[list(range(N_CORES))],
                        ins=[ag_inA.opt()], outs=[h2tabA.opt()])
                    nc.sync.dma_start(h2tab[0:N_CORES * AG_HALF, :], h2tabA[:])

            edge_layer(h1tab, F1, H1, ROW1, adst1, b1s, sink1, src16, mid_hook)

            nc.gpsimd.collective_compute(
                "AllGather", OP.bypass, replica_groups=[list(range(N_CORES))],
                ins=[ag_inB.opt()], outs=[h2tabB.opt()])
            nc.sync.dma_start(h2tab[N_CORES * AG_HALF:2 * N_CORES * AG_HALF, :],
                              h2tabB[:])

            def sink2(b, o2r, ps_epi, epi):
                nc.sync.dma_start(out_d[NPB * b:NPB * (b + 1), :], o2r[:NPB, 0:F2])

            edge_layer(h2tab, F2, 1, ROW2, adst2, b2s, sink2, src16L2)

    nc.compile()
    return nc


def kernel(**inputs) -> np.ndarray:
    import time

    from concourse.bass_utils import run_bass_kernel_spmd

    shared, per_core, tile_ofs, Ttot, Epad = _host_prep(inputs)
    nc = _build_program(tile_ofs, Ttot, Epad)

    in_maps = []
    for c in range(N_CORES):
        m = dict(shared)
        m.update(per_core[c])
        in_maps.append(m)
    res = None
    for attempt in range(3):
        try:
            res = run_bass_kernel_spmd(nc, in_maps, list(range(N_CORES)))
            break
        except Exception:
            if attempt == 2:
                raise
            time.sleep(5)
    out = np.concatenate([res.results[c]["out"] for c in range(N_CORES)], axis=0)
    return np.ascontiguousarray(out.astype(np.float32))
